# revision 68
# baseline (speedup 1.0000x reference)
"""Trainium2 Bass kernel for nn_Autoencoder_8847632630342.

Fake-quantized (int8 per-tensor) 1x1-conv autoencoder:
  9 encoder layers of [quant_conv1x1 -> batchnorm -> quant_relu6] + decoder quant_conv1x1.
Input x [16, 640, 64, 64] f32; output same shape.

Strategy (hardcoded for this problem):
- Pure data parallel: batch 16 -> 2 per core across 8 NeuronCores; pixels
  (2*64*64 = 8192 per core) on the free axis, channels on partitions.
- Exact integer arithmetic: quantized activations/weights are small integers,
  held in bf16 (exact <= 256) and matmul'd on the TensorEngine with fp32 PSUM
  accumulation (bit-exact). Rounding/clipping of the fake-quant ops is done
  with the hardware's saturating round-to-nearest-even dtype converts
  (fp32->int16 / uint8), which match jnp.round/clip semantics exactly.
- Per-tensor quant scales need global maxes: ONE AllReduce(max) per layer of a
  per-channel [C,2] (max, -min) payload of the integer conv accumulator.
  Everything downstream (output quant scale, the post-BN/ReLU6 requant max and
  the next layer's input scale) is derived locally from the global per-channel
  extrema via monotonicity of the elementwise chain.
"""
import os
import sys
import types

sys.path.insert(0, "/opt/trn_rl_repo")

import numpy as np
import ml_dtypes

# Bridge the NTFF profile hook that trn_boot couldn't install (antenv stub in
# this image lacks axon_hooks). Only needed when tracing.
try:
    from trn_agent_boot.trn_boot import _ntff_profile_via_ctypes

    if "antenv.axon_hooks" not in sys.modules:
        _hook = _ntff_profile_via_ctypes("/opt/axon/libaxon_pjrt.so")
        _mod = types.ModuleType("antenv.axon_hooks")
        _mod.get_axon_ntff_profile_hook = lambda: _hook
        _mod.set_axon_ntff_profile_hook = lambda h: None
        sys.modules["antenv.axon_hooks"] = _mod
except Exception:
    pass

import concourse.bass as bass
import concourse.mybir as mybir
import concourse.tile as tile
from concourse import bacc
from concourse import bass_isa
from concourse.bass import ts
from concourse.bass_utils import run_bass_kernel_spmd
from concourse.masks import make_identity

F32 = mybir.dt.float32
BF16 = mybir.dt.bfloat16
I16 = mybir.dt.int16
U8 = mybir.dt.uint8
AX = mybir.AluOpType
ACTF = mybir.ActivationFunctionType
AXL = mybir.AxisListType

N_CORES = 8
B = 16
B_LOC = B // N_CORES           # 2
HW = 64 * 64                   # 4096
PIX = B_LOC * HW               # 8192 pixels per core
FD = 512                       # pixel tile (1 PSUM bank of f32)
NT = PIX // FD                 # 16
BD = 2048                      # batched elementwise pass width
NBD = PIX // BD                # 4
CW = 2048                      # input streaming chunk width (1 MiB DMAs)

ENC_CH = [(640, 128), (128, 128), (128, 128), (128, 128), (128, 8),
          (8, 128), (128, 128), (128, 128), (128, 128)]
DEC_CH = (128, 640)
N_ENC = len(ENC_CH)
RELU_SCALE = np.float32(6.0 / 255.0)   # matches reference f32 value
INV_RS = float(np.float32(1.0) / RELU_SCALE)
C127 = float(np.float32(1.0) / np.float32(127.0))

_BUILD_CACHE = {}
_SW = None  # per-layer weight quant scales (host floats, baked as immediates)


def _host_prep(params):
    """Replicate the reference's f32 weight-side quantization on host."""
    prep = []
    for p in params:
        W = np.asarray(p["W"], np.float32)          # [cout, cin]
        b = np.asarray(p["b"], np.float32)          # [cout]
        m = np.float32(max(np.abs(W).max(), np.float32(1e-8)))
        sw = np.float32(m / np.float32(127.0))
        wq = np.clip(np.round(W / sw), -128, 127).astype(np.float32)
        bw = b.astype(np.float32)
        # channels >128 are laid out [128, groups] (group g = ch 128g..128g+127)
        bw = bw[:, None] if bw.size <= 128 else np.ascontiguousarray(
            bw.reshape(-1, 128).T)
        ent = {
            "sw": float(sw),
            "wT": np.ascontiguousarray(wq.T).astype(ml_dtypes.bfloat16),  # [cin, cout]
            "bw": bw,
        }
        if "gamma" in p:
            g = np.asarray(p["gamma"], np.float32)
            be = np.asarray(p["beta"], np.float32)
            mu = np.asarray(p["mean"], np.float32)
            var = np.asarray(p["var"], np.float32)
            inv = (g / np.sqrt(var + np.float32(1e-5))).astype(np.float32)
            shift = (be - mu * inv).astype(np.float32)
            ent["bninv"] = inv[:, None]
            ent["shift"] = shift[:, None]
        prep.append(ent)
    return prep


def _build():
    nc = bacc.Bacc(num_devices=N_CORES, name="autoenc")

    xd = nc.dram_tensor("x", [B_LOC, 640, 64, 64], F32, kind="ExternalInput")
    outd = nc.dram_tensor("out", [B_LOC, 640, 64, 64], F32, kind="ExternalOutput")
    wT_d, bw_d, inv_d, shift_d = [], [], [], []
    for k in range(N_ENC + 1):
        cin, cout = ENC_CH[k] if k < N_ENC else DEC_CH
        wT_d.append(nc.dram_tensor(f"w{k}", [cin, cout], BF16, kind="ExternalInput"))
        bw_shape = [cout, 1] if cout <= 128 else [128, cout // 128]
        bw_d.append(nc.dram_tensor(f"bw{k}", bw_shape, F32, kind="ExternalInput"))
        if k < N_ENC:
            inv_d.append(nc.dram_tensor(f"inv{k}", [cout, 1], F32, kind="ExternalInput"))
            shift_d.append(nc.dram_tensor(f"sh{k}", [cout, 1], F32, kind="ExternalInput"))

    RG = [list(range(N_CORES))]

    with tile.TileContext(nc) as tc:
        with tc.tile_pool(name="wp", bufs=1) as wp, \
             tc.tile_pool(name="tiny", bufs=4) as tp, \
             tc.tile_pool(name="ps", bufs=8, space="PSUM") as psp, \
             tc.tile_pool(name="dram", bufs=1, space="DRAM") as drp:

            # ---- load weights / per-layer host constants ----
            w_sb, bw_sb, inv_sb, shift_sb = [], [], [], []
            for k in range(N_ENC + 1):
                cin, cout = ENC_CH[k] if k < N_ENC else DEC_CH
                gi = max(1, cin // 128)
                if gi > 1:
                    w = wp.tile([128, gi, cout], BF16, tag=f"w{k}")
                    nc.gpsimd.dma_start(w[:], wT_d[k].rearrange("(g p) o -> p g o", p=128))
                else:
                    w = wp.tile([cin, cout], BF16, tag=f"w{k}")
                    nc.gpsimd.dma_start(w[:], wT_d[k][:])
                w_sb.append(w)
                bw = wp.tile([cout, 1] if cout <= 128 else [128, cout // 128],
                             F32, tag=f"bw{k}")
                nc.gpsimd.dma_start(bw[:], bw_d[k][:])
                bw_sb.append(bw)
                if k < N_ENC:
                    iv = wp.tile([cout, 1], F32, tag=f"iv{k}")
                    nc.gpsimd.dma_start(iv[:], inv_d[k][:])
                    inv_sb.append(iv)
                    sh = wp.tile([cout, 1], F32, tag=f"sh{k}")
                    nc.gpsimd.dma_start(sh[:], shift_d[k][:])
                    shift_sb.append(sh)

            ident = wp.tile([128, 128], F32, tag="ident")
            make_identity(nc, ident[:])

            warm_i = drp.tile([1, 8], F32, tag="warm_i")
            warm_o = drp.tile([1, 8], F32, tag="warm_o")
            nc.gpsimd.collective_compute(
                "AllReduce", AX.max, replica_groups=RG,
                ins=[warm_i.opt()], outs=[warm_o.opt()])
            warm_s = tp.tile([1, 8], F32, tag="warm_s")
            nc.gpsimd.dma_start(warm_s[:], warm_o[:])

            def allreduce_max(pay, P, n, k):
                """AllReduce(max) of an SBUF [P, n] payload across the 8 cores.

                The payload is PE-transposed to [n, P] so the DRAM bounce DMAs
                use n partition descriptors instead of P (P=128 bounces were
                descriptor-latency bound)."""
                pT_ps = psp.tile([n, P], F32, tag="ps")
                nc.tensor.transpose(pT_ps[:], pay[:], ident[:P, :P])
                pT = tp.tile([n, P], F32, tag="ccpT")
                nc.vector.tensor_copy(pT[:], pT_ps[:])
                ci = drp.tile([n, P], F32, tag=f"cci{k}")
                co = drp.tile([n, P], F32, tag=f"cco{k}")
                nc.sync.dma_start(ci[:], pT[:])
                nc.gpsimd.collective_compute(
                    "AllReduce", AX.max, replica_groups=RG,
                    ins=[ci.opt()], outs=[co.opt()])
                gT = tp.tile([n, P], F32, tag="ccgT")
                nc.sync.dma_start(gT[:], co[:])
                g_ps = psp.tile([P, n], F32, tag="ps")
                nc.tensor.transpose(g_ps[:], gT[:], ident[:n, :n])
                g = tp.tile([P, n], F32, tag="ccg")
                nc.vector.tensor_copy(g[:], g_ps[:])
                return g

            def par_max_bcast(v, P, tag):
                """max over partitions [P,1] -> [128,1] broadcast (pad with 0)."""
                if P < 128:
                    pad = tp.tile([128, 1], F32, tag=f"{tag}_pad")
                    nc.vector.memset(pad[:], 0.0)
                    nc.vector.tensor_copy(pad[:P], v[:P])
                    v = pad
                out = tp.tile([128, 1], F32, tag=f"{tag}_par")
                nc.gpsimd.partition_all_reduce(out[:], v[:], 128, bass_isa.ReduceOp.max)
                return out

            def recip(v, P, tag):
                r = tp.tile([P, 1], F32, tag=tag)
                nc.vector.reciprocal(r[:], v[:P])
                return r

            def ts_op(in_, s1, op1v, P, tag, dtype=F32, s2=None, op2=None):
                o = tp.tile([P, 1], dtype, tag=tag)
                nc.vector.tensor_scalar(o[:], in_[:P], s1, s2, op1v,
                                        op2 if op2 is not None else AX.bypass)
                return o

            def tt_op(a, bv, opv, P, tag):
                o = tp.tile([P, 1], F32, tag=tag)
                nc.vector.tensor_tensor(o[:], a[:P], bv[:P], opv)
                return o

            # ---------------------------------------------------------
            # Encoder layer body (shared). k=0 recomputes matmuls to
            # avoid holding a [128, PIX] f32 accumulator next to xi0.
            # ---------------------------------------------------------
            def enc_layer(k, xi, s_in, accp, rotp, pre_A=None):
                cin, cout = ENC_CH[k]
                Gi = max(1, cin // 128)
                Po = min(cout, 128)

                def mm(t):
                    ps = psp.tile([Po, FD], F32, tag="ps")
                    for g in range(Gi):
                        rhs = xi[:, g, ts(t, FD)] if Gi > 1 else xi[:, ts(t, FD)]
                        nc.tensor.matmul(ps[:], w_sb[k][:, g] if Gi > 1 else w_sb[k][:],
                                         rhs, start=(g == 0), stop=(g == Gi - 1))
                    return ps

                # ---- pre-barrier constants (only need s_in) ----
                sw = float(np.float32(_SW[k]))
                s_unit = ts_op(s_in, sw, AX.mult, Po, "sunit")
                inv_sb_v = recip(s_unit, Po, "invsb")
                bq_i = ts_op(bw_sb[k], inv_sb_v[:], AX.mult, Po, "bqi", dtype=I16)
                bq = tp.tile([Po, 1], F32, tag="bqf")
                nc.vector.tensor_copy(bq[:], bq_i[:])
                Bv = ts_op(shift_sb[k], INV_RS, AX.mult, Po, "Bv")

                if pre_A is None:
                    acc = accp.tile([Po, PIX], F32, tag="acc")
                    cmax_c = tp.tile([Po, NT], F32, tag="cmaxc")
                    cmin_c = tp.tile([Po, NBD], F32, tag="cminc")
                    for t in range(NT):
                        ps = mm(t)
                        nc.vector.tensor_scalar(acc[:, ts(t, FD)], ps[:], 0.0, None,
                                                AX.add, AX.max,
                                                accum_out=cmax_c[:, t:t + 1])
                    for q in range(NBD):
                        nc.vector.tensor_scalar(acc[:, ts(q, BD)], acc[:, ts(q, BD)],
                                                0.0, None, AX.add, AX.min,
                                                accum_out=cmin_c[:, q:q + 1])
                else:
                    acc, cmax_c, cmin_c = pre_A

                pay = tp.tile([Po, 2], F32, tag="pay")
                nc.vector.tensor_reduce(pay[:, 0:1], cmax_c[:], axis=AXL.X, op=AX.max)
                mn = tp.tile([Po, 1], F32, tag="mn")
                nc.vector.tensor_reduce(mn[:], cmin_c[:], axis=AXL.X, op=AX.min)
                nc.vector.tensor_scalar(pay[:, 1:2], mn[:], -1.0, None, AX.mult)

                gl = allreduce_max(pay, Po, 2, f"L{k}")
                cmax_g = gl[:, 0:1]
                cmin_g = ts_op(gl[:, 1:2], -1.0, AX.mult, Po, "cming")

                # ---- post-barrier scale derivation (tiny, [Po,1]) ----
                vmax = tt_op(cmax_g, bq, AX.add, Po, "vmax")
                nvmin = tt_op(gl[:, 1:2], bq, AX.subtract, Po, "nvmin")
                mc = tt_op(vmax, nvmin, AX.max, Po, "mc")
                Mv = par_max_bcast(mc, Po, f"M{k}")
                c1 = ts_op(recip(Mv, 128, f"rM{k}"), 127.0, AX.mult, 128, "c1")
                bqc1 = tt_op(bq, c1, AX.mult, Po, "bqc1")
                syv = tt_op(s_unit, Mv, AX.mult, Po, "syv")
                syv = ts_op(syv, C127, AX.mult, Po, "syv2")
                Av = ts_op(syv, inv_sb[k][:Po], AX.mult, Po, "Av",
                           s2=INV_RS, op2=AX.mult)

                # tiny twins on ScalarE (must exactly match bulk P1)
                qmax_i = tp.tile([Po, 1], I16, tag="qmaxi")
                nc.scalar.activation(qmax_i[:], cmax_g[:], ACTF.Identity,
                                     bias=bqc1[:], scale=c1[:Po])
                qmin_i = tp.tile([Po, 1], I16, tag="qmini")
                nc.scalar.activation(qmin_i[:], cmin_g[:], ACTF.Identity,
                                     bias=bqc1[:], scale=c1[:Po])
                # twins of P2 on VectorE (i16/u8 inputs upcast internally)
                riA = ts_op(qmax_i, Av[:], AX.mult, Po, "riA", dtype=U8,
                            s2=Bv[:], op2=AX.add)
                riB = ts_op(qmin_i, Av[:], AX.mult, Po, "riB", dtype=U8,
                            s2=Bv[:], op2=AX.add)
                rimax = tt_op(riA, riB, AX.max, Po, "rimax")
                rim = par_max_bcast(rimax, Po, f"rim{k}")
                c2 = ts_op(recip(rim, 128, f"rrim{k}"), 127.0, AX.mult, 128, "c2")
                s_next = ts_op(rim, float(RELU_SCALE), AX.mult, 128, "snext0")
                s_next = ts_op(s_next, C127, AX.mult, 128, "snext")

                # ---- bulk chain (batched BD): P1 qy(i16) [ACT], P2 ri(u8) [DVE],
                #      P3 xq(i16) [GpSimd], P4 xi_next(bf16) [DVE] ----
                xi_n = xip.tile([Po, PIX], BF16, tag="xi")
                for q in range(NBD):
                    qy = rotp.tile([Po, BD], I16, tag="qy")
                    nc.scalar.activation(qy[:], acc[:, ts(q, BD)],
                                         ACTF.Identity, bias=bqc1[:],
                                         scale=c1[:Po])
                    ri = rotp.tile([Po, BD], U8, tag="ri")
                    nc.vector.tensor_scalar(ri[:], qy[:], Av[:], Bv[:],
                                            AX.mult, AX.add)
                    # P3 writes back into the qy tile (values no longer needed)
                    nc.scalar.activation(qy[:], ri[:], ACTF.Identity,
                                         bias=0.0, scale=c2[:Po])
                    nc.vector.tensor_copy(xi_n[:, ts(q, BD)], qy[:])
                return xi_n, s_next

            with tc.tile_pool(name="accp", bufs=1) as accp, \
                 tc.tile_pool(name="xi", bufs=2) as xip, \
                 tc.tile_pool(name="rot", bufs=2) as rotp:

                # =================================================
                # Input: absmax (round 1, 2 MiB chunks), AllReduce,
                # quantize (round 2, pixel-major 1 MiB chunks)
                # =================================================
                with tc.tile_pool(name="xch4", bufs=3) as x4p:
                    xabs_cols = tp.tile([128, 5, B_LOC], F32, tag="xabs")
                    di = 0
                    for g in range(5):
                        for b in range(B_LOC):
                            ch = x4p.tile([128, HW], F32, tag="xchunk4")
                            di += 1
                            nc.sync.dma_start(
                                ch[:],
                                xd[b, g * 128:(g + 1) * 128]
                                .rearrange("c h w -> c (h w)"))
                            nc.vector.tensor_reduce(
                                xabs_cols[:, g, b:b + 1],
                                ch[:], axis=AXL.X, op=AX.max,
                                apply_absolute_value=True)
                    xam = tp.tile([128, 5], F32, tag="xam")
                    nc.vector.tensor_reduce(xam[:], xabs_cols[:], axis=AXL.X,
                                            op=AX.max)
                    xg = allreduce_max(xam, 128, 5, "L0in")
                    xgm = tp.tile([128, 1], F32, tag="xgm")
                    nc.vector.tensor_reduce(xgm[:], xg[:], axis=AXL.X, op=AX.max)
                    M0 = par_max_bcast(xgm, 128, "M0")
                    c0 = ts_op(recip(M0, 128, "rM0"), 127.0, AX.mult, 128, "c0")
                    s_in = ts_op(M0, C127, AX.mult, 128, "sin0")

                with tc.tile_pool(name="xi0p", bufs=1) as xi0p:
                    NCH = HW // CW  # chunks per (b, g)
                    with tc.tile_pool(name="xch", bufs=2) as xchp, \
                         tc.tile_pool(name="xq0r", bufs=1) as xq0p:
                        # pixel-major fill so layer-0 matmuls on early pixel
                        # tiles can start while later chunks still stream in
                        xi0 = xi0p.tile([128, 5, PIX], BF16, tag="xi0")
                        acc0 = accp.tile([128, PIX], F32, tag="acc")
                        cmax0 = tp.tile([128, NT], F32, tag="cmaxc")
                        cmin0 = tp.tile([128, NBD], F32, tag="cminc")
                        for b in range(B_LOC):
                            for c2i in range(NCH):
                                for g in range(5):
                                    ch = xchp.tile([128, CW], F32, tag="xchunk")
                                    di += 1
                                    nc.sync.dma_start(
                                        ch[:],
                                        xd[b, g * 128:(g + 1) * 128]
                                        .rearrange("c h w -> c (h w)")[:, ts(c2i, CW)])
                                    q = xq0p.tile([128, CW], I16, tag="xq0")
                                    nc.vector.tensor_scalar(q[:], ch[:], c0[:], None,
                                                            AX.mult)
                                    nc.vector.tensor_copy(
                                        xi0[:, g, b * HW + c2i * CW:
                                            b * HW + (c2i + 1) * CW], q[:])
                                # this 2048-pixel window is complete across all
                                # 5 groups: emit layer-0 matmuls + extrema now
                                # so they hide under the remaining DMA stream
                                qb = b * NCH + c2i          # BD batch index
                                for t in range(qb * (BD // FD),
                                               (qb + 1) * (BD // FD)):
                                    ps = psp.tile([128, FD], F32, tag="ps")
                                    for g in range(5):
                                        nc.tensor.matmul(
                                            ps[:], w_sb[0][:, g],
                                            xi0[:, g, ts(t, FD)],
                                            start=(g == 0), stop=(g == 4))
                                    nc.vector.tensor_scalar(
                                        acc0[:, ts(t, FD)], ps[:], 0.0, None,
                                        AX.add, AX.max,
                                        accum_out=cmax0[:, t:t + 1])
                                nc.vector.tensor_scalar(
                                    acc0[:, ts(qb, BD)], acc0[:, ts(qb, BD)],
                                    0.0, None, AX.add, AX.min,
                                    accum_out=cmin0[:, qb:qb + 1])

                    # Layer 0 barrier + P-phase (A-phase already emitted above)
                    xi, s_in = enc_layer(0, xi0, s_in, accp, rotp,
                                         pre_A=(acc0, cmax0, cmin0))

                # Layers 1..8
                for k in range(1, N_ENC):
                    xi, s_in = enc_layer(k, xi, s_in, accp, rotp)

                # =================================================
                # Decoder (128 -> 640), recompute scheme
                # =================================================
                kd = N_ENC
                with tc.tile_pool(name="stg", bufs=3) as stgp:
                    swd = float(np.float32(_SW[kd]))
                    # bias (needs only s_in) before round 1
                    s_unit_d = ts_op(s_in, swd, AX.mult, 128, "sunitd")
                    inv_sbd = recip(s_unit_d, 128, "invsbd")
                    bqd_f = []
                    for g in range(5):
                        bq_i = ts_op(bw_sb[kd][:, g:g + 1], inv_sbd[:], AX.mult,
                                     128, f"dbqi{g}", dtype=I16)
                        bqf = tp.tile([128, 1], F32, tag=f"dbqf{g}")
                        nc.vector.tensor_copy(bqf[:], bq_i[:])
                        bqd_f.append(bqf)

                    # round 1: |acc+bq| via ScalarE Abs, absmax accum on VectorE
                    dabs = tp.tile([128, 5, NBD], F32, tag="dabs")
                    for g in range(5):
                        for q in range(NBD):
                            thr = stgp.tile([128, BD], F32, tag="dthrow")
                            for s in range(BD // FD):
                                t = q * (BD // FD) + s
                                ps = psp.tile([128, FD], F32, tag="ps")
                                nc.tensor.matmul(
                                    ps[:], w_sb[kd][:, g * 128:(g + 1) * 128],
                                    xi[:, ts(t, FD)], start=True, stop=True)
                                nc.scalar.activation(thr[:, ts(s, FD)], ps[:],
                                                     ACTF.Abs, bias=bqd_f[g][:],
                                                     scale=1.0)
                            nc.vector.tensor_scalar(thr[:], thr[:], 0.0, None,
                                                    AX.add, AX.max,
                                                    accum_out=dabs[:, g, q:q + 1])
                    dpay = tp.tile([128, 5], F32, tag="dpay")
                    nc.vector.tensor_reduce(dpay[:], dabs[:], axis=AXL.X, op=AX.max)
                    dgl = allreduce_max(dpay, 128, 5, "Ld")
                    dmc = tp.tile([128, 1], F32, tag="dmc")
                    nc.vector.tensor_reduce(dmc[:], dgl[:], axis=AXL.X, op=AX.max)
                    Md = par_max_bcast(dmc, 128, "Md")
                    cd = ts_op(recip(Md, 128, "rMd"), 127.0, AX.mult, 128, "cd")
                    syd = tt_op(s_unit_d, Md, AX.mult, 128, "syd")
                    syd = ts_op(syd, C127, AX.mult, 128, "syd2")
                    bqcd = [tt_op(bqd_f[g], cd, AX.mult, 128, f"bqcd{g}")
                            for g in range(5)]

                    # round 2: recompute + quantize + dequantize + store
                    NH = HW // BD
                    di = 0
                    for b in range(B_LOC):
                        for g in range(5):
                            for h in range(NH):
                                od = stgp.tile([128, BD], F32, tag="odec")
                                qd = stgp.tile([128, BD], I16, tag="qdec")
                                for t8 in range(BD // FD):
                                    t = b * (HW // FD) + h * (BD // FD) + t8
                                    ps = psp.tile([128, FD], F32, tag="ps")
                                    nc.tensor.matmul(
                                        ps[:], w_sb[kd][:, g * 128:(g + 1) * 128],
                                        xi[:, ts(t, FD)], start=True, stop=True)
                                    nc.scalar.activation(qd[:, ts(t8, FD)], ps[:],
                                                         ACTF.Identity,
                                                         bias=bqcd[g][:], scale=cd[:])
                                nc.vector.tensor_scalar(od[:], qd[:], syd[:],
                                                        None, AX.mult)
                                di += 1
                                nc.sync.dma_start(
                                    outd[b, g * 128:(g + 1) * 128]
                                    .rearrange("c h w -> c (h w)")[:, ts(h, BD)],
                                    od[:])

    nc.compile()
    return nc


def kernel(x, params):
    global _SW
    x = np.asarray(x, np.float32)
    prep = _host_prep(params)
    _SW = [p["sw"] for p in prep]

    if "nc" not in _BUILD_CACHE:
        _BUILD_CACHE["nc"] = _build()
    nc = _BUILD_CACHE["nc"]

    base = {}
    for k, p in enumerate(prep):
        base[f"w{k}"] = p["wT"]
        base[f"bw{k}"] = p["bw"]
        if "bninv" in p:
            base[f"inv{k}"] = p["bninv"]
            base[f"sh{k}"] = p["shift"]

    in_maps = []
    for c in range(N_CORES):
        m = dict(base)
        m["x"] = np.ascontiguousarray(x[c * B_LOC:(c + 1) * B_LOC])
        in_maps.append(m)

    trace = os.environ.get("AE_TRACE", "0") == "1"
    res = run_bass_kernel_spmd(nc, in_maps, core_ids=list(range(N_CORES)),
                               trace=trace)
    if trace and res.exec_time_ns is not None:
        print(f"HW exec time: {res.exec_time_ns} ns")
        kernel.last_exec_time_ns = res.exec_time_ns
        kernel.last_trace = res.instructions_and_trace
    out = np.concatenate([res.results[c]["out"] for c in range(N_CORES)], axis=0)
    return out


# revision 70
# speedup vs baseline: 1.0251x; 1.0251x over previous
"""Trainium2 Bass kernel for nn_Autoencoder_8847632630342.

Fake-quantized (int8 per-tensor) 1x1-conv autoencoder:
  9 encoder layers of [quant_conv1x1 -> batchnorm -> quant_relu6] + decoder quant_conv1x1.
Input x [16, 640, 64, 64] f32; output same shape.

Strategy (hardcoded for this problem):
- Pure data parallel: batch 16 -> 2 per core across 8 NeuronCores; pixels
  (2*64*64 = 8192 per core) on the free axis, channels on partitions.
- Exact integer arithmetic: quantized activations/weights are small integers,
  held in bf16 (exact <= 256) and matmul'd on the TensorEngine with fp32 PSUM
  accumulation (bit-exact). Rounding/clipping of the fake-quant ops is done
  with the hardware's saturating round-to-nearest-even dtype converts
  (fp32->int16 / uint8), which match jnp.round/clip semantics exactly.
- Per-tensor quant scales need global maxes: ONE AllReduce(max) per layer of a
  per-channel [C,2] (max, -min) payload of the integer conv accumulator.
  Everything downstream (output quant scale, the post-BN/ReLU6 requant max and
  the next layer's input scale) is derived locally from the global per-channel
  extrema via monotonicity of the elementwise chain.
"""
import os
import sys
import types

sys.path.insert(0, "/opt/trn_rl_repo")

import numpy as np
import ml_dtypes

# Bridge the NTFF profile hook that trn_boot couldn't install (antenv stub in
# this image lacks axon_hooks). Only needed when tracing.
try:
    from trn_agent_boot.trn_boot import _ntff_profile_via_ctypes

    if "antenv.axon_hooks" not in sys.modules:
        _hook = _ntff_profile_via_ctypes("/opt/axon/libaxon_pjrt.so")
        _mod = types.ModuleType("antenv.axon_hooks")
        _mod.get_axon_ntff_profile_hook = lambda: _hook
        _mod.set_axon_ntff_profile_hook = lambda h: None
        sys.modules["antenv.axon_hooks"] = _mod
except Exception:
    pass

import concourse.bass as bass
import concourse.mybir as mybir
import concourse.tile as tile
from concourse import bacc
from concourse import bass_isa
from concourse.bass import ts
from concourse.bass_utils import run_bass_kernel_spmd
from concourse.masks import make_identity

F32 = mybir.dt.float32
BF16 = mybir.dt.bfloat16
I16 = mybir.dt.int16
U8 = mybir.dt.uint8
AX = mybir.AluOpType
ACTF = mybir.ActivationFunctionType
AXL = mybir.AxisListType

N_CORES = 8
B = 16
B_LOC = B // N_CORES           # 2
HW = 64 * 64                   # 4096
PIX = B_LOC * HW               # 8192 pixels per core
FD = 512                       # pixel tile (1 PSUM bank of f32)
NT = PIX // FD                 # 16
BD = 2048                      # batched elementwise pass width
NBD = PIX // BD                # 4
CW = 2048                      # input streaming chunk width (1 MiB DMAs)

ENC_CH = [(640, 128), (128, 128), (128, 128), (128, 128), (128, 8),
          (8, 128), (128, 128), (128, 128), (128, 128)]
DEC_CH = (128, 640)
N_ENC = len(ENC_CH)
RELU_SCALE = np.float32(6.0 / 255.0)   # matches reference f32 value
INV_RS = float(np.float32(1.0) / RELU_SCALE)
C127 = float(np.float32(1.0) / np.float32(127.0))

_BUILD_CACHE = {}
_SW = None  # per-layer weight quant scales (host floats, baked as immediates)


def _host_prep(params):
    """Replicate the reference's f32 weight-side quantization on host."""
    prep = []
    for p in params:
        W = np.asarray(p["W"], np.float32)          # [cout, cin]
        b = np.asarray(p["b"], np.float32)          # [cout]
        m = np.float32(max(np.abs(W).max(), np.float32(1e-8)))
        sw = np.float32(m / np.float32(127.0))
        wq = np.clip(np.round(W / sw), -128, 127).astype(np.float32)
        bw = b.astype(np.float32)
        # channels >128 are laid out [128, groups] (group g = ch 128g..128g+127)
        bw = bw[:, None] if bw.size <= 128 else np.ascontiguousarray(
            bw.reshape(-1, 128).T)
        ent = {
            "sw": float(sw),
            "wT": np.ascontiguousarray(wq.T).astype(ml_dtypes.bfloat16),  # [cin, cout]
            "bw": bw,
        }
        if "gamma" in p:
            g = np.asarray(p["gamma"], np.float32)
            be = np.asarray(p["beta"], np.float32)
            mu = np.asarray(p["mean"], np.float32)
            var = np.asarray(p["var"], np.float32)
            inv = (g / np.sqrt(var + np.float32(1e-5))).astype(np.float32)
            shift = (be - mu * inv).astype(np.float32)
            ent["bninv"] = inv[:, None]
            ent["shift"] = shift[:, None]
        prep.append(ent)
    return prep


def _build():
    nc = bacc.Bacc(num_devices=N_CORES, name="autoenc")

    xd = nc.dram_tensor("x", [B_LOC, 640, 64, 64], F32, kind="ExternalInput")
    outd = nc.dram_tensor("out", [B_LOC, 640, 64, 64], F32, kind="ExternalOutput")
    wT_d, bw_d, inv_d, shift_d = [], [], [], []
    for k in range(N_ENC + 1):
        cin, cout = ENC_CH[k] if k < N_ENC else DEC_CH
        wT_d.append(nc.dram_tensor(f"w{k}", [cin, cout], BF16, kind="ExternalInput"))
        bw_shape = [cout, 1] if cout <= 128 else [128, cout // 128]
        bw_d.append(nc.dram_tensor(f"bw{k}", bw_shape, F32, kind="ExternalInput"))
        if k < N_ENC:
            inv_d.append(nc.dram_tensor(f"inv{k}", [cout, 1], F32, kind="ExternalInput"))
            shift_d.append(nc.dram_tensor(f"sh{k}", [cout, 1], F32, kind="ExternalInput"))

    RG = [list(range(N_CORES))]

    with tile.TileContext(nc) as tc:
        with tc.tile_pool(name="wp", bufs=1) as wp, \
             tc.tile_pool(name="tiny", bufs=4) as tp, \
             tc.tile_pool(name="ps", bufs=8, space="PSUM") as psp, \
             tc.tile_pool(name="dram", bufs=1, space="DRAM") as drp:

            # ---- load weights / per-layer host constants ----
            w_sb, bw_sb, inv_sb, shift_sb = [], [], [], []
            for k in range(N_ENC + 1):
                cin, cout = ENC_CH[k] if k < N_ENC else DEC_CH
                gi = max(1, cin // 128)
                if gi > 1:
                    w = wp.tile([128, gi, cout], BF16, tag=f"w{k}")
                    nc.scalar.dma_start(w[:], wT_d[k].rearrange("(g p) o -> p g o", p=128))
                else:
                    w = wp.tile([cin, cout], BF16, tag=f"w{k}")
                    nc.scalar.dma_start(w[:], wT_d[k][:])
                w_sb.append(w)
                bw = wp.tile([cout, 1] if cout <= 128 else [128, cout // 128],
                             F32, tag=f"bw{k}")
                nc.gpsimd.dma_start(bw[:], bw_d[k][:])
                bw_sb.append(bw)
                if k < N_ENC:
                    iv = wp.tile([cout, 1], F32, tag=f"iv{k}")
                    nc.gpsimd.dma_start(iv[:], inv_d[k][:])
                    inv_sb.append(iv)
                    sh = wp.tile([cout, 1], F32, tag=f"sh{k}")
                    nc.gpsimd.dma_start(sh[:], shift_d[k][:])
                    shift_sb.append(sh)

            ident = wp.tile([128, 128], F32, tag="ident")
            make_identity(nc, ident[:])

            warm_i = drp.tile([1, 8], F32, tag="warm_i")
            warm_o = drp.tile([1, 8], F32, tag="warm_o")
            nc.gpsimd.collective_compute(
                "AllReduce", AX.max, replica_groups=RG,
                ins=[warm_i.opt()], outs=[warm_o.opt()])
            warm_s = tp.tile([1, 8], F32, tag="warm_s")
            nc.gpsimd.dma_start(warm_s[:], warm_o[:])

            def allreduce_max(pay, P, n, k):
                """AllReduce(max) of an SBUF [P, n] payload across the 8 cores.

                The payload is PE-transposed to [n, P] so the DRAM bounce DMAs
                use n partition descriptors instead of P (P=128 bounces were
                descriptor-latency bound)."""
                pT_ps = psp.tile([n, P], F32, tag="ps")
                nc.tensor.transpose(pT_ps[:], pay[:], ident[:P, :P])
                pT = tp.tile([n, P], F32, tag="ccpT")
                nc.vector.tensor_copy(pT[:], pT_ps[:])
                ci = drp.tile([n, P], F32, tag=f"cci{k}")
                co = drp.tile([n, P], F32, tag=f"cco{k}")
                nc.sync.dma_start(ci[:], pT[:])
                nc.gpsimd.collective_compute(
                    "AllReduce", AX.max, replica_groups=RG,
                    ins=[ci.opt()], outs=[co.opt()])
                gT = tp.tile([n, P], F32, tag="ccgT")
                nc.sync.dma_start(gT[:], co[:])
                g_ps = psp.tile([P, n], F32, tag="ps")
                nc.tensor.transpose(g_ps[:], gT[:], ident[:n, :n])
                g = tp.tile([P, n], F32, tag="ccg")
                nc.vector.tensor_copy(g[:], g_ps[:])
                return g

            def par_max_bcast(v, P, tag):
                """max over partitions [P,1] -> [128,1] broadcast (pad with 0)."""
                if P < 128:
                    pad = tp.tile([128, 1], F32, tag=f"{tag}_pad")
                    nc.vector.memset(pad[:], 0.0)
                    nc.vector.tensor_copy(pad[:P], v[:P])
                    v = pad
                out = tp.tile([128, 1], F32, tag=f"{tag}_par")
                nc.gpsimd.partition_all_reduce(out[:], v[:], 128, bass_isa.ReduceOp.max)
                return out

            def recip(v, P, tag):
                r = tp.tile([P, 1], F32, tag=tag)
                nc.vector.reciprocal(r[:], v[:P])
                return r

            def ts_op(in_, s1, op1v, P, tag, dtype=F32, s2=None, op2=None):
                o = tp.tile([P, 1], dtype, tag=tag)
                nc.vector.tensor_scalar(o[:], in_[:P], s1, s2, op1v,
                                        op2 if op2 is not None else AX.bypass)
                return o

            def tt_op(a, bv, opv, P, tag):
                o = tp.tile([P, 1], F32, tag=tag)
                nc.vector.tensor_tensor(o[:], a[:P], bv[:P], opv)
                return o

            # ---------------------------------------------------------
            # Encoder layer body (shared). k=0 recomputes matmuls to
            # avoid holding a [128, PIX] f32 accumulator next to xi0.
            # ---------------------------------------------------------
            def enc_layer(k, xi, s_in, accp, rotp):
                cin, cout = ENC_CH[k]
                Gi = max(1, cin // 128)
                Po = min(cout, 128)
                first = accp is None

                def mm(t):
                    ps = psp.tile([Po, FD], F32, tag="ps")
                    for g in range(Gi):
                        rhs = xi[:, g, ts(t, FD)] if Gi > 1 else xi[:, ts(t, FD)]
                        nc.tensor.matmul(ps[:], w_sb[k][:, g] if Gi > 1 else w_sb[k][:],
                                         rhs, start=(g == 0), stop=(g == Gi - 1))
                    return ps

                # ---- pre-barrier constants (only need s_in) ----
                sw = float(np.float32(_SW[k]))
                s_unit = ts_op(s_in, sw, AX.mult, Po, "sunit")
                inv_sb_v = recip(s_unit, Po, "invsb")
                bq_i = ts_op(bw_sb[k], inv_sb_v[:], AX.mult, Po, "bqi", dtype=I16)
                bq = tp.tile([Po, 1], F32, tag="bqf")
                nc.vector.tensor_copy(bq[:], bq_i[:])
                Bv = ts_op(shift_sb[k], INV_RS, AX.mult, Po, "Bv")

                acc = accp.tile([Po, PIX], F32, tag="acc")
                cmax_c = tp.tile([Po, NT], F32, tag="cmaxc")
                cmin_c = tp.tile([Po, NBD], F32, tag="cminc")
                for t in range(NT):
                    ps = mm(t)
                    nc.vector.tensor_scalar(acc[:, ts(t, FD)], ps[:], 0.0, None,
                                            AX.add, AX.max,
                                            accum_out=cmax_c[:, t:t + 1])
                for q in range(NBD):
                    nc.vector.tensor_scalar(acc[:, ts(q, BD)], acc[:, ts(q, BD)],
                                            0.0, None, AX.add, AX.min,
                                            accum_out=cmin_c[:, q:q + 1])

                pay = tp.tile([Po, 2], F32, tag="pay")
                nc.vector.tensor_reduce(pay[:, 0:1], cmax_c[:], axis=AXL.X, op=AX.max)
                mn = tp.tile([Po, 1], F32, tag="mn")
                nc.vector.tensor_reduce(mn[:], cmin_c[:], axis=AXL.X, op=AX.min)
                nc.vector.tensor_scalar(pay[:, 1:2], mn[:], -1.0, None, AX.mult)

                gl = allreduce_max(pay, Po, 2, f"L{k}")
                cmax_g = gl[:, 0:1]
                cmin_g = ts_op(gl[:, 1:2], -1.0, AX.mult, Po, "cming")

                # ---- post-barrier scale derivation (tiny, [Po,1]) ----
                vmax = tt_op(cmax_g, bq, AX.add, Po, "vmax")
                nvmin = tt_op(gl[:, 1:2], bq, AX.subtract, Po, "nvmin")
                mc = tt_op(vmax, nvmin, AX.max, Po, "mc")
                Mv = par_max_bcast(mc, Po, f"M{k}")
                c1 = ts_op(recip(Mv, 128, f"rM{k}"), 127.0, AX.mult, 128, "c1")
                bqc1 = tt_op(bq, c1, AX.mult, Po, "bqc1")
                syv = tt_op(s_unit, Mv, AX.mult, Po, "syv")
                syv = ts_op(syv, C127, AX.mult, Po, "syv2")
                Av = ts_op(syv, inv_sb[k][:Po], AX.mult, Po, "Av",
                           s2=INV_RS, op2=AX.mult)

                # tiny twins on ScalarE (must exactly match bulk P1)
                qmax_i = tp.tile([Po, 1], I16, tag="qmaxi")
                nc.scalar.activation(qmax_i[:], cmax_g[:], ACTF.Identity,
                                     bias=bqc1[:], scale=c1[:Po])
                qmin_i = tp.tile([Po, 1], I16, tag="qmini")
                nc.scalar.activation(qmin_i[:], cmin_g[:], ACTF.Identity,
                                     bias=bqc1[:], scale=c1[:Po])
                # twins of P2 on VectorE (i16/u8 inputs upcast internally)
                riA = ts_op(qmax_i, Av[:], AX.mult, Po, "riA", dtype=U8,
                            s2=Bv[:], op2=AX.add)
                riB = ts_op(qmin_i, Av[:], AX.mult, Po, "riB", dtype=U8,
                            s2=Bv[:], op2=AX.add)
                rimax = tt_op(riA, riB, AX.max, Po, "rimax")
                rim = par_max_bcast(rimax, Po, f"rim{k}")
                c2 = ts_op(recip(rim, 128, f"rrim{k}"), 127.0, AX.mult, 128, "c2")
                s_next = ts_op(rim, float(RELU_SCALE), AX.mult, 128, "snext0")
                s_next = ts_op(s_next, C127, AX.mult, 128, "snext")

                # ---- bulk chain (batched BD): P1 qy(i16) [ACT], P2 ri(u8) [DVE],
                #      P3 xq(i16) [GpSimd], P4 xi_next(bf16) [DVE] ----
                xi_n = xip.tile([Po, PIX], BF16, tag="xi")
                for q in range(NBD):
                    qy = rotp.tile([Po, BD], I16, tag="qy")
                    nc.scalar.activation(qy[:], acc[:, ts(q, BD)],
                                         ACTF.Identity, bias=bqc1[:],
                                         scale=c1[:Po])
                    ri = rotp.tile([Po, BD], U8, tag="ri")
                    nc.vector.tensor_scalar(ri[:], qy[:], Av[:], Bv[:],
                                            AX.mult, AX.add)
                    # P3 writes back into the qy tile (values no longer needed)
                    nc.scalar.activation(qy[:], ri[:], ACTF.Identity,
                                         bias=0.0, scale=c2[:Po])
                    nc.vector.tensor_copy(xi_n[:, ts(q, BD)], qy[:])
                return xi_n, s_next

            with tc.tile_pool(name="accp", bufs=1) as accp, \
                 tc.tile_pool(name="xi", bufs=2) as xip, \
                 tc.tile_pool(name="rot", bufs=2) as rotp:

                # =================================================
                # Input: absmax (round 1, 2 MiB chunks), AllReduce,
                # quantize (round 2, pixel-major 1 MiB chunks)
                # =================================================
                with tc.tile_pool(name="xch4", bufs=3) as x4p:
                    xabs_cols = tp.tile([128, 5, B_LOC], F32, tag="xabs")
                    di = 0
                    for g in range(5):
                        for b in range(B_LOC):
                            ch = x4p.tile([128, HW], F32, tag="xchunk4")
                            di += 1
                            nc.sync.dma_start(
                                ch[:],
                                xd[b, g * 128:(g + 1) * 128]
                                .rearrange("c h w -> c (h w)"))
                            nc.vector.tensor_reduce(
                                xabs_cols[:, g, b:b + 1],
                                ch[:], axis=AXL.X, op=AX.max,
                                apply_absolute_value=True)
                    xam = tp.tile([128, 5], F32, tag="xam")
                    nc.vector.tensor_reduce(xam[:], xabs_cols[:], axis=AXL.X,
                                            op=AX.max)
                    xg = allreduce_max(xam, 128, 5, "L0in")
                    xgm = tp.tile([128, 1], F32, tag="xgm")
                    nc.vector.tensor_reduce(xgm[:], xg[:], axis=AXL.X, op=AX.max)
                    M0 = par_max_bcast(xgm, 128, "M0")
                    c0 = ts_op(recip(M0, 128, "rM0"), 127.0, AX.mult, 128, "c0")
                    s_in = ts_op(M0, C127, AX.mult, 128, "sin0")

                with tc.tile_pool(name="xi0p", bufs=1) as xi0p:
                    NCH = HW // CW  # chunks per (b, g)
                    with tc.tile_pool(name="xch", bufs=2) as xchp, \
                         tc.tile_pool(name="xq0r", bufs=1) as xq0p:
                        # pixel-major fill so layer-0 matmuls on early pixel
                        # tiles can start while later chunks still stream in
                        xi0 = xi0p.tile([128, 5, PIX], BF16, tag="xi0")
                        for b in range(B_LOC):
                            for c2i in range(NCH):
                                for g in range(5):
                                    ch = xchp.tile([128, CW], F32, tag="xchunk")
                                    di += 1
                                    nc.sync.dma_start(
                                        ch[:],
                                        xd[b, g * 128:(g + 1) * 128]
                                        .rearrange("c h w -> c (h w)")[:, ts(c2i, CW)])
                                    q = xq0p.tile([128, CW], I16, tag="xq0")
                                    nc.vector.tensor_scalar(q[:], ch[:], c0[:], None,
                                                            AX.mult)
                                    nc.vector.tensor_copy(
                                        xi0[:, g, b * HW + c2i * CW:
                                            b * HW + (c2i + 1) * CW], q[:])

                    # Layer 0 (xi0 freed afterwards)
                    xi, s_in = enc_layer(0, xi0, s_in, accp, rotp)

                # Layers 1..8
                for k in range(1, N_ENC):
                    xi, s_in = enc_layer(k, xi, s_in, accp, rotp)

                # =================================================
                # Decoder (128 -> 640), recompute scheme
                # =================================================
                kd = N_ENC
                with tc.tile_pool(name="stg", bufs=3) as stgp:
                    swd = float(np.float32(_SW[kd]))
                    # bias (needs only s_in) before round 1
                    s_unit_d = ts_op(s_in, swd, AX.mult, 128, "sunitd")
                    inv_sbd = recip(s_unit_d, 128, "invsbd")
                    bqd_f = []
                    for g in range(5):
                        bq_i = ts_op(bw_sb[kd][:, g:g + 1], inv_sbd[:], AX.mult,
                                     128, f"dbqi{g}", dtype=I16)
                        bqf = tp.tile([128, 1], F32, tag=f"dbqf{g}")
                        nc.vector.tensor_copy(bqf[:], bq_i[:])
                        bqd_f.append(bqf)

                    # round 1: |acc+bq| via ScalarE Abs, absmax accum on VectorE
                    dabs = tp.tile([128, 5, NBD], F32, tag="dabs")
                    for g in range(5):
                        for q in range(NBD):
                            thr = stgp.tile([128, BD], F32, tag="dthrow")
                            for s in range(BD // FD):
                                t = q * (BD // FD) + s
                                ps = psp.tile([128, FD], F32, tag="ps")
                                nc.tensor.matmul(
                                    ps[:], w_sb[kd][:, g * 128:(g + 1) * 128],
                                    xi[:, ts(t, FD)], start=True, stop=True)
                                nc.scalar.activation(thr[:, ts(s, FD)], ps[:],
                                                     ACTF.Abs, bias=bqd_f[g][:],
                                                     scale=1.0)
                            nc.vector.tensor_scalar(thr[:], thr[:], 0.0, None,
                                                    AX.add, AX.max,
                                                    accum_out=dabs[:, g, q:q + 1])
                    dpay = tp.tile([128, 5], F32, tag="dpay")
                    nc.vector.tensor_reduce(dpay[:], dabs[:], axis=AXL.X, op=AX.max)
                    dgl = allreduce_max(dpay, 128, 5, "Ld")
                    dmc = tp.tile([128, 1], F32, tag="dmc")
                    nc.vector.tensor_reduce(dmc[:], dgl[:], axis=AXL.X, op=AX.max)
                    Md = par_max_bcast(dmc, 128, "Md")
                    cd = ts_op(recip(Md, 128, "rMd"), 127.0, AX.mult, 128, "cd")
                    syd = tt_op(s_unit_d, Md, AX.mult, 128, "syd")
                    syd = ts_op(syd, C127, AX.mult, 128, "syd2")
                    bqcd = [tt_op(bqd_f[g], cd, AX.mult, 128, f"bqcd{g}")
                            for g in range(5)]

                    # round 2: recompute + quantize + dequantize + store
                    NH = HW // BD
                    di = 0
                    for b in range(B_LOC):
                        for g in range(5):
                            for h in range(NH):
                                od = stgp.tile([128, BD], F32, tag="odec")
                                qd = stgp.tile([128, BD], I16, tag="qdec")
                                for t8 in range(BD // FD):
                                    t = b * (HW // FD) + h * (BD // FD) + t8
                                    ps = psp.tile([128, FD], F32, tag="ps")
                                    nc.tensor.matmul(
                                        ps[:], w_sb[kd][:, g * 128:(g + 1) * 128],
                                        xi[:, ts(t, FD)], start=True, stop=True)
                                    nc.scalar.activation(qd[:, ts(t8, FD)], ps[:],
                                                         ACTF.Identity,
                                                         bias=bqcd[g][:], scale=cd[:])
                                nc.vector.tensor_scalar(od[:], qd[:], syd[:],
                                                        None, AX.mult)
                                di += 1
                                nc.sync.dma_start(
                                    outd[b, g * 128:(g + 1) * 128]
                                    .rearrange("c h w -> c (h w)")[:, ts(h, BD)],
                                    od[:])

    nc.compile()
    return nc


def kernel(x, params):
    global _SW
    x = np.asarray(x, np.float32)
    prep = _host_prep(params)
    _SW = [p["sw"] for p in prep]

    if "nc" not in _BUILD_CACHE:
        _BUILD_CACHE["nc"] = _build()
    nc = _BUILD_CACHE["nc"]

    base = {}
    for k, p in enumerate(prep):
        base[f"w{k}"] = p["wT"]
        base[f"bw{k}"] = p["bw"]
        if "bninv" in p:
            base[f"inv{k}"] = p["bninv"]
            base[f"sh{k}"] = p["shift"]

    in_maps = []
    for c in range(N_CORES):
        m = dict(base)
        m["x"] = np.ascontiguousarray(x[c * B_LOC:(c + 1) * B_LOC])
        in_maps.append(m)

    trace = os.environ.get("AE_TRACE", "0") == "1"
    res = run_bass_kernel_spmd(nc, in_maps, core_ids=list(range(N_CORES)),
                               trace=trace)
    if trace and res.exec_time_ns is not None:
        print(f"HW exec time: {res.exec_time_ns} ns")
        kernel.last_exec_time_ns = res.exec_time_ns
        kernel.last_trace = res.instructions_and_trace
    out = np.concatenate([res.results[c]["out"] for c in range(N_CORES)], axis=0)
    return out
